# revision 2
# baseline (speedup 1.0000x reference)
"""Mixture-of-Experts (top-2 of 8 experts, erf-GELU FFN) on 8 Trainium2
NeuronCores.

Sharding: pure tensor-parallel over the FFN dim — every core holds ALL 8
experts but only an F/8 = 512 slice of W1/W2.  The dispatched token set is
identical on all cores (the full top-2 dispatch, C = T*top_k = 4096 token
slots grouped by expert), so per-core capacity is exactly C/1 with ZERO
load-imbalance padding (an expert-parallel split pays max-over-groups
padding; this pays none).  Each core computes a partial y over its F-slice;
the host sums the 8 partials, applies the combine weights, and scatter-adds
into the output.  b2 is applied on the host.

Host side (inside kernel()): router softmax + top-2 + renormalized combine
weights, token dispatch (gather per expert slot), combine (scatter-add of
the F-slice partial sums, scaled by the combine weight — the device never
sees wg).

Device side (Bass/Tile SPMD), NB_F = 4 F-blocks per core:
  MM1 (chunk-major, fb inner): for each token chunk of its expert slot,
      h[fb] = gelu(sum_db w1[s,fb,db]^T @ x[chunk] + b1[s,fb])
  MM2 (dt-major):  y[dt] partial = sum_fb w2[s,fb,dt]^T @ h[fb]
bf16 matmuls, fp32 PSUM accumulation, bf16 partial-y output.

Layouts shipped per core (P=128 partitions, C = 4096):
  x   [P, NB_D*C] bf16  chunk-blocked x^T: chunk k holds
                        [p, db, c] = x[off_k+c, db*128+p]
  w1  [NSLOT, P, NB_F, NB_D, P]  [s,p,fb,db,m] = W1[e_s][db*128+p, Foff+fb*128+m]
  w2  [NB_D, P, NSLOT, NB_F, P]  [dt,k,s,fb,m] = W2[e_s][Foff+fb*128+k, dt*128+m]
  b1  [P, NSLOT*NB_F] f32        [p, s*NB_F+fb] = b1[e_s][Foff+fb*128+p]
  out [P, NB_D, C] bf16          partial y^T (unweighted)

MM1 is chunk-major so x streams through a 5-deep ring (x never needs to be
fully resident) and each chunk holds only 4 PSUM banks (fb0-3); w1 arrives
as one DMA per slot (slot 0 split per-fb so the first matmul unblocks
early).  Queues: sync = w1 then w2 slabs (ring, just-in-time during MM2);
scalar = x ring then batched out slices; gpsimd = b1.

The kernel opens with a short (~3.4 us) dummy-matmul warmup on a memset
scratch tile: the PE's HAM clock gate holds the array at 1.2 GHz until it
has seen a full ~3.4 us busy window, and the DMA ramp can't feed real
matmuls that early anyway.
"""

import numpy as np
import ml_dtypes

P = 128
N_CORES = 8
D, F, E = 1024, 4096, 8
NB_D = D // P
TOP_K = 2

NSLOT = E                 # all experts on every core
NSPLIT = N_CORES          # F split 8 ways
FS = F // NSPLIT          # 512
NB_F = FS // P            # 4

MAX_CHUNK = 512           # PSUM bank = 512 fp32 columns
HEAD_CHUNK = 128          # small first chunk so MM1 starts during the DMA ramp
NWARM = 8                 # ~3.4 us of cold N=512 dummies: one full HAM window
X_BUFS = 5                # x ring depth (chunks)

_cache = {}
_last_in_maps = None


def _chunk_plan(S):
    """MM1 chunk grid: slot-major, per-slot even chunks <= MAX_CHUNK.

    Slot 0 leads with a small head chunk: its x block + first w1 tile are
    the critical DMA prefix before the first real matmul can issue.
    Returns list of (slot, off, cn, xoff) and C.
    """
    chunks = []
    off = 0
    xoff = 0
    for s, sz in enumerate(S):
        sizes = []
        rem = sz
        if s == 0 and HEAD_CHUNK + P < rem <= HEAD_CHUNK + MAX_CHUNK:
            sizes.append(HEAD_CHUNK)
            rem -= HEAD_CHUNK
        n_ch = max(1, -(-rem // MAX_CHUNK))
        c0 = 0
        for i in range(n_ch):
            cn = (rem - c0 + (n_ch - 1 - i)) // (n_ch - i)
            sizes.append(cn)
            c0 += cn
        c0 = 0
        for cn in sizes:
            chunks.append((s, off + c0, cn, xoff))
            xoff += NB_D * cn
            c0 += cn
        off += sz
    return chunks, off


def _chunk_plan2(S):
    """MM2's chunk grid: per-slot even chunks, no head split.  Slots are
    ordered by descending count, so the grid naturally ends on the smallest
    chunk — the kernel's exposed tail."""
    chunks = []
    off = 0
    for s, sz in enumerate(S):
        n_ch = max(1, -(-sz // MAX_CHUNK))
        c0 = 0
        for i in range(n_ch):
            cn = (sz - c0 + (n_ch - 1 - i)) // (n_ch - i)
            chunks.append((s, off + c0, cn))
            c0 += cn
        off += sz
    return chunks


def _build(S):
    """Build + compile the per-core SPMD Bass program for slot sizes S."""
    from concourse import bacc
    import concourse.tile as tile
    import concourse.mybir as mybir

    chunks, C = _chunk_plan(S)
    chunks2 = _chunk_plan2(S)
    bf16 = mybir.dt.bfloat16
    f32 = mybir.dt.float32
    GELU = mybir.ActivationFunctionType.Gelu

    nc = bacc.Bacc(None, target_bir_lowering=False)
    x_d = nc.dram_tensor("x", [P, NB_D * C], bf16, kind="ExternalInput")
    w1_d = nc.dram_tensor("w1", [NSLOT, P, NB_F, NB_D, P], bf16, kind="ExternalInput")
    w2_d = nc.dram_tensor("w2", [NB_D, P, NSLOT, NB_F, P], bf16, kind="ExternalInput")
    b1_d = nc.dram_tensor("b1", [P, NSLOT * NB_F], f32, kind="ExternalInput")
    out_d = nc.dram_tensor("out", [P, NB_D, C], bf16, kind="ExternalOutput")

    with tile.TileContext(nc) as tc:
        with (
            tc.tile_pool(name="const", bufs=1) as const,
            tc.tile_pool(name="xp", bufs=X_BUFS) as xp,
            tc.tile_pool(name="w1p", bufs=1) as w1p,
            tc.tile_pool(name="w2p", bufs=4) as w2p,
            tc.tile_pool(name="ps1", bufs=5, space="PSUM") as ps1p,
            tc.tile_pool(name="ps2", bufs=3, space="PSUM") as ps2p,
            tc.tile_pool(name="outp", bufs=2) as outp,
        ):
            b1_t = const.tile([P, NSLOT * NB_F], f32)
            h_t = const.tile([P, NB_F, C], bf16)
            scr_t = const.tile([P, MAX_CHUNK], bf16)

            nc.gpsimd.dma_start(b1_t[:], b1_d[:])

            # PE warm-up: the HAM clock gate runs the PE at 1.2 GHz until it
            # has seen a full ~3.4 us busy window; the DMA ramp can't feed
            # real matmuls during that window anyway.  8 cold N=512 dummies
            # span exactly ~3.4 us.  Results land in a PSUM bank MM2 resets.
            nc.vector.memset(scr_t[:], 0.0)
            wps = ps2p.tile([P, MAX_CHUNK], f32, name="ps2", tag="ps2")
            for i in range(NWARM):
                nc.tensor.matmul(
                    wps[:],
                    lhsT=scr_t[:, :P],
                    rhs=scr_t[:],
                    start=(i == 0),
                    stop=(i == NWARM - 1),
                )

            # x rides the scalar queue (HWDGE) as a ring; w1/w2 ride the
            # sync queue.  Both queues drain in FIFO order, so each stream
            # is hand-ordered in consumption order and they share the HBM
            # bandwidth at packet granularity during the head.
            x_ts = [None] * len(chunks)

            def _x_dma(k, n_sub=1):
                s, off, cn, xoff = chunks[k]
                t = xp.tile([P, NB_D * MAX_CHUNK], bf16, name="x", tag="x")
                x_ts[k] = t
                step = NB_D // n_sub * cn
                for i in range(n_sub):
                    nc.scalar.dma_start(
                        t[:, i * step : (i + 1) * step],
                        x_d[:, xoff + i * step : xoff + (i + 1) * step],
                    )

            _x_dma(0, n_sub=2)
            for k in range(1, min(X_BUFS, len(chunks))):
                _x_dma(k)

            # w1: slot 0 per-fb (the first matmuls' exact needs), the rest
            # as one contiguous 1 MB slab per slot.
            w1_t = w1p.tile([P, NSLOT, NB_F, NB_D, P], bf16, name="w1", tag="w1")
            for fb in range(NB_F):
                nc.sync.dma_start(w1_t[:, 0, fb], w1_d[0][:, fb])
            for s in range(1, NSLOT):
                nc.sync.dma_start(w1_t[:, s], w1_d[s])

            # ---- MM1, chunk-major (fb inner): 4 PSUM banks per chunk, x
            # consumed exactly once per chunk -> ring.
            for k, (s, off, cn, xoff) in enumerate(chunks):
                x_t = x_ts[k]
                for fb in range(NB_F):
                    ps = ps1p.tile([P, MAX_CHUNK], f32)
                    for db in range(NB_D):
                        nc.tensor.matmul(
                            ps[:, :cn],
                            lhsT=w1_t[:, s, fb, db, :],
                            rhs=x_t[:, db * cn : (db + 1) * cn],
                            start=(db == 0),
                            stop=(db == NB_D - 1),
                        )
                    nc.scalar.activation(
                        h_t[:, fb, off : off + cn],
                        ps[:, :cn],
                        GELU,
                        bias=b1_t[:, s * NB_F + fb : s * NB_F + fb + 1],
                    )
                if k + X_BUFS < len(chunks):
                    _x_dma(k + X_BUFS)

            # ---- MM2, dt-major: w2 dt-slabs stream just-in-time from a
            # ring; out slices DMA per half-dt so only the last small chunk
            # is the exposed tail.
            n2 = len(chunks2)
            for dt in range(NB_D):
                w2_t = w2p.tile([P, NSLOT, NB_F, P], bf16, name="w2d", tag="w2d")
                nc.sync.dma_start(w2_t[:], w2_d[dt])
                o_t = outp.tile([P, C], bf16, name="o", tag="o")
                if dt == NB_D - 1:
                    cuts = [n2 // 2, n2 - 1, n2]
                else:
                    cuts = [n2 // 2, n2]
                lo = 0
                for cut in cuts:
                    for s, off, cn in chunks2[lo:cut]:
                        ps = ps2p.tile([P, MAX_CHUNK], f32, name="ps2", tag="ps2")
                        for fb in range(NB_F):
                            nc.tensor.matmul(
                                ps[:, :cn],
                                lhsT=w2_t[:, s, fb, :],
                                rhs=h_t[:, fb, off : off + cn],
                                start=(fb == 0),
                                stop=(fb == NB_F - 1),
                            )
                        nc.vector.tensor_copy(o_t[:, off : off + cn], ps[:, :cn])
                    a = chunks2[lo][1]
                    b = chunks2[cut - 1][1] + chunks2[cut - 1][2]
                    nc.scalar.dma_start(out_d[:, dt, a:b], o_t[:, a:b])
                    lo = cut

    nc.compile()
    return nc


def _route(x, W_router):
    """Top-2 routing, replicating jax softmax/top_k/renorm semantics."""
    T = x.shape[0]
    logits = x @ np.asarray(W_router, np.float32)
    m = logits.max(axis=1, keepdims=True)
    ex = np.exp(logits - m, dtype=np.float32)
    probs = ex / ex.sum(axis=1, keepdims=True, dtype=np.float32)
    r = np.arange(T)
    i1 = probs.argmax(axis=1)
    masked = probs.copy()
    masked[r, i1] = -np.inf
    i2 = masked.argmax(axis=1)
    p1 = probs[r, i1]
    p2 = probs[r, i2]
    s = p1 + p2
    return i1, i2, p1 / s, p2 / s


def kernel(hidden_states, W_router, W1, b1, W2, b2):
    from concourse.bass_utils import run_bass_kernel_spmd

    B, S_, D_ = hidden_states.shape
    T = B * S_
    x = np.ascontiguousarray(np.asarray(hidden_states, np.float32).reshape(T, D_))

    i1, i2, w1c, w2c = _route(x, W_router)

    idxs, wgts = [], []
    for e in range(E):
        sel1 = i1 == e
        sel2 = i2 == e
        idx = np.nonzero(sel1 | sel2)[0]
        w = np.where(sel1[idx], w1c[idx], w2c[idx]).astype(np.float32)
        idxs.append(idx)
        wgts.append(w)

    counts = [len(ix) for ix in idxs]
    order = list(np.argsort(-np.asarray(counts), kind="stable"))
    S = [counts[e] for e in order]
    chunks, C = _chunk_plan(S)
    offs = np.concatenate([[0], np.cumsum(S)])[:NSLOT]

    key = tuple(S)
    if key not in _cache:
        _cache[key] = _build(S)
    nc = _cache[key]

    bf16 = ml_dtypes.bfloat16
    xb = x.astype(bf16)
    W1f = np.asarray(W1, np.float32)
    W2f = np.asarray(W2, np.float32)
    b1f = np.asarray(b1, np.float32)

    # dispatched x, grouped by expert slot (identical on every core)
    xg = np.empty((C, D), bf16)
    for s, e in enumerate(order):
        xg[offs[s] : offs[s] + counts[e]] = xb[idxs[e]]
    x_arr = np.empty((P, NB_D * C), bf16)
    for s, off, cn, xoff in chunks:
        x_arr[:, xoff : xoff + NB_D * cn] = (
            xg[off : off + cn]
            .T.reshape(NB_D, P, cn)
            .transpose(1, 0, 2)
            .reshape(P, NB_D * cn)
        )

    in_maps = [None] * N_CORES
    for j in range(N_CORES):
        foff = j * FS
        w1e = np.ascontiguousarray(
            W1f[order][:, :, foff : foff + FS]
            .astype(bf16)
            .reshape(NSLOT, NB_D, P, NB_F, P)
            .transpose(0, 2, 3, 1, 4)
        )
        w2e = np.ascontiguousarray(
            W2f[order][:, foff : foff + FS, :]
            .astype(bf16)
            .reshape(NSLOT, NB_F, P, NB_D, P)
            .transpose(3, 2, 0, 1, 4)
        )
        b1e = np.ascontiguousarray(
            b1f[order][:, foff : foff + FS]
            .reshape(NSLOT, NB_F, P)
            .transpose(2, 0, 1)
            .reshape(P, NSLOT * NB_F)
        )
        in_maps[j] = {"x": x_arr, "w1": w1e, "w2": w2e, "b1": b1e}

    global _last_in_maps
    _last_in_maps = in_maps

    res = run_bass_kernel_spmd(nc, in_maps, core_ids=list(range(N_CORES)))

    acc = np.zeros((P, NB_D, C), np.float32)
    for j in range(N_CORES):
        acc += np.asarray(res.results[j]["out"], np.float32)

    out = np.zeros((T, D), np.float32)
    b2f = np.asarray(b2, np.float32)
    for s, e in enumerate(order):
        n = counts[e]
        y = acc[:, :, offs[s] : offs[s] + n].transpose(2, 1, 0).reshape(n, D)
        out[idxs[e]] += wgts[e][:, None] * y
        if b2f[e].any():
            out[idxs[e]] += wgts[e][:, None] * b2f[e][None, :]
    return out.reshape(B, S_, D_).astype(np.float32)
